# revision 5
# baseline (speedup 1.0000x reference)
"""Self-contained Trainium2 Bass kernel for nn_CharModel (dense transformer
forward: embed -> single-head causal attention -> vocab projection).

Distribution over 8 NeuronCores:
  - sequence-parallel attention: core c owns tokens [c*512, (c+1)*512)
  - vocab-parallel logits: core c owns padded-vocab columns [c*6400, (c+1)*6400)
  - attention outputs are exchanged with 4 chunked bf16 AllGathers
All matmuls run in bf16 with fp32 PSUM accumulation; softmax stats fp32.
"""
import numpy as np

import concourse.bass as bass
import concourse.mybir as mybir
import concourse.tile as tile
from concourse import bacc
from concourse.bass_utils import run_bass_kernel_spmd
from concourse.masks import make_identity

P = 128
N_TOK = 4096
D = 1024
VOCAB = 50257
NC = 8
VPAD_TOT = 51200  # 50257 padded up to 400*128
VSH = VPAD_TOT // NC  # 6400 per-core vocab shard
OWN = N_TOK // NC  # 512 own tokens
IBLK = OWN // P  # 4 own row-blocks
KT = D // P  # 8 contraction tiles
OT = D // P  # 8 output-feature tiles
CHUNKS = N_TOK // 512  # 8 projection chunks (512 tokens each)
JB = N_TOK // 512  # 8 key strips of 512
JB2 = N_TOK // P  # 32 key tiles of 128
SCALE = 1.0 / 32.0  # 1/sqrt(D)

F32 = mybir.dt.float32
BF16 = mybir.dt.bfloat16
I32 = mybir.dt.int32

# logits v-strips within the 6400-wide shard: 12 x 512 + 1 x 256
VSTRIPS = [(i * 512, 512) for i in range(12)] + [(6144, 256)]


def build(nc: bass.Bass):
    tok = nc.dram_tensor("tok", [N_TOK], I32, kind="ExternalInput")
    qtok = nc.dram_tensor("qtok", [OWN], I32, kind="ExternalInput")
    E = nc.dram_tensor("E", [VOCAB, D], F32, kind="ExternalInput")
    WqT = nc.dram_tensor("WqT", [D, D], F32, kind="ExternalInput")
    WkT = nc.dram_tensor("WkT", [D, D], F32, kind="ExternalInput")
    WvT = nc.dram_tensor("WvT", [D, D], F32, kind="ExternalInput")
    bq = nc.dram_tensor("bq", [D], F32, kind="ExternalInput")
    bk = nc.dram_tensor("bk", [D], F32, kind="ExternalInput")
    bv = nc.dram_tensor("bv", [D], F32, kind="ExternalInput")
    WpT = nc.dram_tensor("WpT", [D, VSH], F32, kind="ExternalInput")
    bp = nc.dram_tensor("bp", [VSH], F32, kind="ExternalInput")
    # ridx_sh[r, jb] = global_row(r) - jb*512, fp32
    ridx_sh = nc.dram_tensor("ridx_sh", [OWN, JB], F32, kind="ExternalInput")
    logits = nc.dram_tensor("logits", [N_TOK, VSH], F32, kind="ExternalOutput")

    with tile.TileContext(nc) as tc:
        with (
            tc.tile_pool(name="const", bufs=1) as const,
            tc.tile_pool(name="dram", bufs=1, space="DRAM") as dram,
        ):
            ident = const.tile([P, P], BF16)
            make_identity(nc, ident[:])

            bv_bc = const.tile([P, D], F32)
            nc.sync.dma_start(bv_bc[:], bv.ap()[None, :].to_broadcast([P, D]))

            bq_t = const.tile([P, OT], F32)
            nc.sync.dma_start(bq_t[:], bq.ap().rearrange("(ot p) -> p ot", p=P))
            bk_t = const.tile([P, OT], F32)
            nc.sync.dma_start(bk_t[:], bk.ap().rearrange("(ot p) -> p ot", p=P))

            rsh = const.tile([P, IBLK, JB], F32)
            nc.sync.dma_start(
                rsh[:], ridx_sh.ap().rearrange("(ib p) jb -> p ib jb", p=P)
            )

            jidx0 = const.tile([P, 512], F32)
            ji = const.tile([P, 512], I32)
            nc.gpsimd.iota(ji[:], pattern=[[1, 512]], base=0, channel_multiplier=0)
            nc.vector.tensor_copy(out=jidx0[:], in_=ji[:])

            tok_sb = const.tile([P, N_TOK // P], I32)
            nc.sync.dma_start(tok_sb[:], tok.ap().rearrange("(g p) -> p g", p=P))
            qtok_sb = const.tile([P, OWN // P], I32)
            nc.sync.dma_start(qtok_sb[:], qtok.ap().rearrange("(g p) -> p g", p=P))

            # DRAM scratch
            Kscr = dram.tile([CHUNKS, P, OT, 512], BF16)
            Vscr = dram.tile([JB2, P, D], BF16)
            oTb = [dram.tile([P, KT, P], BF16, name=f"oTb{q}") for q in range(IBLK)]
            gat = [
                dram.tile([NC, P, KT, P], BF16, name=f"gat{q}") for q in range(IBLK)
            ]

            # ---------------- gather + transpose helper ----------------
            def gather_xT(pool, pspool, idx_sb, g0, ngroups, tag):
                """gather token groups [g0, g0+ngroups) -> xT [P, KT, ngroups*P] bf16"""
                xT = pool.tile([P, KT, ngroups * P], BF16, tag=f"xT_{tag}")
                for g in range(ngroups):
                    xg = pool.tile([P, D], F32, tag="xg")
                    nc.gpsimd.indirect_dma_start(
                        out=xg[:],
                        out_offset=None,
                        in_=E.ap(),
                        in_offset=bass.IndirectOffsetOnAxis(
                            ap=idx_sb[:, g0 + g : g0 + g + 1], axis=0
                        ),
                    )
                    xb = pool.tile([P, D], BF16, tag="xb")
                    nc.vector.tensor_copy(out=xb[:], in_=xg[:])
                    for kt in range(KT):
                        pst = pspool.tile([P, P], BF16, tag="ptr")
                        nc.tensor.transpose(
                            pst[:], xb[:, kt * P : (kt + 1) * P], ident[:]
                        )
                        nc.vector.tensor_copy(
                            out=xT[:, kt, g * P : (g + 1) * P], in_=pst[:]
                        )
                return xT

            def load_w(pool, dramt, tag):
                wb = pool.tile([P, KT, D], BF16, tag=f"wb_{tag}")
                for half in range(4):
                    wf = pool.tile([P, KT, D // 4], F32, tag="wf")
                    nc.sync.dma_start(
                        wf[:],
                        dramt.ap().rearrange("(kt p) o -> p kt o", p=P)[
                            :, :, half * (D // 4) : (half + 1) * (D // 4)
                        ],
                    )
                    nc.vector.tensor_copy(
                        out=wb[:, :, half * (D // 4) : (half + 1) * (D // 4)],
                        in_=wf[:],
                    )
                return wb

            # ---------------- phase Q: own-token Q projection ----------------
            qT_pool = tc.alloc_tile_pool(name="qT_keep", bufs=1)
            qT = qT_pool.tile([P, OT, OWN], BF16)
            with (
                tc.tile_pool(name="sbq", bufs=2) as sbq,
                tc.tile_pool(name="psq_tr", bufs=2, space="PSUM") as psq_tr,
                tc.tile_pool(name="psq_pp", bufs=4, space="PSUM") as psq_pp,
            ):
                wq_b = load_w(sbq, WqT, "wq")
                xqT = gather_xT(sbq, psq_tr, qtok_sb, 0, OWN // P, "q")
                for ot in range(OT):
                    pp = psq_pp.tile([P, OWN], F32, tag="pp")
                    for kt in range(KT):
                        nc.tensor.matmul(
                            pp[:],
                            lhsT=wq_b[:, kt, ot * P : (ot + 1) * P],
                            rhs=xqT[:, kt, :],
                            start=(kt == 0),
                            stop=(kt == KT - 1),
                        )
                    nc.vector.tensor_scalar(
                        out=qT[:, ot, :],
                        in0=pp[:],
                        scalar1=bq_t[:, ot : ot + 1],
                        scalar2=SCALE,
                        op0=mybir.AluOpType.add,
                        op1=mybir.AluOpType.mult,
                    )

            # ---------------- phase KV: full K/V projections, spill to DRAM ----
            with (
                tc.tile_pool(name="sbkv", bufs=2) as sbkv,
                tc.tile_pool(name="pskv_tr", bufs=2, space="PSUM") as pskv_tr,
                tc.tile_pool(name="pskv_pp", bufs=2, space="PSUM") as pskv_pp,
                tc.tile_pool(name="pskv_pv", bufs=2, space="PSUM") as pskv_pv,
            ):
                wk_b = load_w(sbkv, WkT, "wk")
                wv_b = load_w(sbkv, WvT, "wv")
                for ch in range(CHUNKS):
                    xT = gather_xT(sbkv, pskv_tr, tok_sb, ch * 4, 4, "kv")
                    # K^T chunk -> Kscr[ch]
                    for ot in range(OT):
                        pk = pskv_pp.tile([P, 512], F32, tag="pp")
                        for kt in range(KT):
                            nc.tensor.matmul(
                                pk[:],
                                lhsT=wk_b[:, kt, ot * P : (ot + 1) * P],
                                rhs=xT[:, kt, :],
                                start=(kt == 0),
                                stop=(kt == KT - 1),
                            )
                        ke = sbkv.tile([P, 512], BF16, tag="ke")
                        nc.vector.tensor_scalar(
                            out=ke[:],
                            in0=pk[:],
                            scalar1=bk_t[:, ot : ot + 1],
                            scalar2=None,
                            op0=mybir.AluOpType.add,
                        )
                        nc.sync.dma_start(Kscr[ch, :, ot, :], ke[:])
                    # V natural chunk -> Vscr[ch*4 + tb]
                    for tb in range(4):
                        pv = pskv_pv.tile([P, D], F32, tag="pv")
                        for kt in range(KT):
                            nc.tensor.matmul(
                                pv[:, 0:512],
                                lhsT=xT[:, kt, tb * P : (tb + 1) * P],
                                rhs=wv_b[:, kt, 0:512],
                                start=(kt == 0),
                                stop=(kt == KT - 1),
                            )
                        for kt in range(KT):
                            nc.tensor.matmul(
                                pv[:, 512:1024],
                                lhsT=xT[:, kt, tb * P : (tb + 1) * P],
                                rhs=wv_b[:, kt, 512:1024],
                                start=(kt == 0),
                                stop=(kt == KT - 1),
                            )
                        ve = sbkv.tile([P, D], BF16, tag="ve")
                        nc.vector.tensor_copy(out=ve[:], in_=pv[:])
                        nc.sync.dma_start(Vscr[ch * 4 + tb, :, :], ve[:])

            # ---------------- phase WPT load (overlaps with attention) --------
            wp_pool = tc.alloc_tile_pool(name="wp_keep", bufs=1)
            wp_b = wp_pool.tile([P, KT, VSH], BF16)
            bp_pool = tc.alloc_tile_pool(name="bp_keep", bufs=1)
            bp_bc = bp_pool.tile([P, VSH], F32)
            with tc.tile_pool(name="sbwp", bufs=2) as sbwp:
                nc.sync.dma_start(bp_bc[:], bp.ap()[None, :].to_broadcast([P, VSH]))
                for kt in range(KT):
                    for half in range(4):
                        v0 = half * (VSH // 4)
                        v1 = (half + 1) * (VSH // 4)
                        wpf = sbwp.tile([P, VSH // 4], F32, tag="wpf")
                        nc.sync.dma_start(
                            wpf[:],
                            WpT.ap().rearrange("(kt p) v -> p kt v", p=P)[
                                :, kt, v0:v1
                            ],
                        )
                        nc.vector.tensor_copy(out=wp_b[:, kt, v0:v1], in_=wpf[:])

            # ---------------- phase attention (own rows) ----------------------
            with (
                tc.tile_pool(name="sbat", bufs=2) as sbat,
                tc.tile_pool(name="psat_sc", bufs=3, space="PSUM") as ps_sc,
                tc.tile_pool(name="psat_av", bufs=1, space="PSUM") as ps_av,
                tc.tile_pool(name="psat_tr", bufs=2, space="PSUM") as ps_tr,
            ):
                for ib in range(IBLK):
                    a_row = sbat.tile([P, N_TOK], BF16, tag="a_row")
                    for jb in range(JB):
                        kjb = sbat.tile([P, OT, 512], BF16, tag="kjb")
                        nc.sync.dma_start(kjb[:], Kscr[jb, :, :, :])
                        ps = ps_sc.tile([P, 512], F32, tag="sc")
                        for ot in range(OT):
                            nc.tensor.matmul(
                                ps[:],
                                lhsT=qT[:, ot, ib * P : (ib + 1) * P],
                                rhs=kjb[:, ot, :],
                                start=(ot == 0),
                                stop=(ot == OT - 1),
                            )
                        astr = a_row[:, jb * 512 : (jb + 1) * 512]
                        nc.scalar.activation(
                            astr, ps[:], mybir.ActivationFunctionType.Exp
                        )
                        # multiply by causal mask: (jidx0 <= ridx - jb*512) * exp
                        nc.vector.scalar_tensor_tensor(
                            out=astr,
                            in0=jidx0[:],
                            scalar=rsh[:, ib, jb : jb + 1],
                            in1=astr,
                            op0=mybir.AluOpType.is_le,
                            op1=mybir.AluOpType.mult,
                        )
                    dsum = sbat.tile([P, 1], F32, tag="dsum")
                    nc.vector.tensor_reduce(
                        out=dsum[:],
                        in_=a_row[:],
                        axis=mybir.AxisListType.X,
                        op=mybir.AluOpType.add,
                    )
                    rden = sbat.tile([P, 1], F32, tag="rden")
                    nc.vector.reciprocal(rden[:], dsum[:])

                    pav = ps_av.tile([P, D], F32, tag="av")
                    for j2 in range(JB2):
                        pat = ps_tr.tile([P, P], BF16, tag="tr")
                        nc.tensor.transpose(
                            pat[:], a_row[:, j2 * P : (j2 + 1) * P], ident[:]
                        )
                        at = sbat.tile([P, P], BF16, tag="at")
                        nc.vector.tensor_copy(out=at[:], in_=pat[:])
                        vj = sbat.tile([P, D], BF16, tag="vj")
                        nc.sync.dma_start(vj[:], Vscr[j2, :, :])
                        nc.tensor.matmul(
                            pav[:, 0:512],
                            lhsT=at[:],
                            rhs=vj[:, 0:512],
                            start=(j2 == 0),
                            stop=(j2 == JB2 - 1),
                        )
                        nc.tensor.matmul(
                            pav[:, 512:1024],
                            lhsT=at[:],
                            rhs=vj[:, 512:1024],
                            start=(j2 == 0),
                            stop=(j2 == JB2 - 1),
                        )
                    o_bf = sbat.tile([P, D], BF16, tag="o_bf")
                    nc.vector.scalar_tensor_tensor(
                        out=o_bf[:],
                        in0=pav[:],
                        scalar=rden[:, :1],
                        in1=bv_bc[:],
                        op0=mybir.AluOpType.mult,
                        op1=mybir.AluOpType.add,
                    )
                    oT = sbat.tile([P, KT, P], BF16, tag="oT")
                    for kt in range(KT):
                        pot = ps_tr.tile([P, P], BF16, tag="tr")
                        nc.tensor.transpose(
                            pot[:], o_bf[:, kt * P : (kt + 1) * P], ident[:]
                        )
                        nc.vector.tensor_copy(out=oT[:, kt, :], in_=pot[:])
                    nc.sync.dma_start(oTb[ib][:], oT[:])
                    nc.gpsimd.collective_compute(
                        "AllGather",
                        mybir.AluOpType.bypass,
                        replica_groups=[list(range(NC))],
                        ins=[oTb[ib].opt()],
                        outs=[gat[ib].opt()],
                    )

            # ---------------- phase logits ------------------------------------
            with (
                tc.tile_pool(name="sblg", bufs=3) as sblg,
                tc.tile_pool(name="pslg", bufs=6, space="PSUM") as pslg,
            ):
                for q in range(IBLK):
                    for c in range(NC):
                        ibg = c * IBLK + q  # global row-block
                        lt = sblg.tile([P, KT, P], BF16, tag="lt")
                        nc.sync.dma_start(lt[:], gat[q][c, :, :, :])
                        for v0, vw in VSTRIPS:
                            pl = pslg.tile([P, 512], F32, tag="lg")
                            for kt in range(KT):
                                nc.tensor.matmul(
                                    pl[:, :vw],
                                    lhsT=lt[:, kt, :],
                                    rhs=wp_b[:, kt, v0 : v0 + vw],
                                    start=(kt == 0),
                                    stop=(kt == KT - 1),
                                )
                            lo = sblg.tile([P, 512], F32, tag="lo")
                            nc.vector.tensor_tensor(
                                out=lo[:, :vw],
                                in0=pl[:, :vw],
                                in1=bp_bc[:, v0 : v0 + vw],
                                op=mybir.AluOpType.add,
                            )
                            nc.sync.dma_start(
                                logits.ap()[
                                    ibg * P : (ibg + 1) * P, v0 : v0 + vw
                                ],
                                lo[:, :vw],
                            )
            bp_pool.release()
            wp_pool.release()
            qT_pool.release()
    return nc


def _prep_inputs(inputs):
    """Host-side shard prep: slicing, transposes, padding only."""
    tokens = np.ascontiguousarray(np.asarray(inputs["tokens"]).astype(np.int32))
    E = np.asarray(inputs["E"], dtype=np.float32)
    WqT = np.ascontiguousarray(np.asarray(inputs["Wq"], np.float32).T)
    WkT = np.ascontiguousarray(np.asarray(inputs["Wk"], np.float32).T)
    WvT = np.ascontiguousarray(np.asarray(inputs["Wv"], np.float32).T)
    Wp = np.asarray(inputs["Wp"], np.float32)
    WpT_pad = np.zeros((D, VPAD_TOT), np.float32)
    WpT_pad[:, :VOCAB] = Wp.T
    bp_pad = np.zeros((VPAD_TOT,), np.float32)
    bp_pad[:VOCAB] = np.asarray(inputs["bp"], np.float32)

    in_maps = []
    for c in range(NC):
        rows = np.arange(c * OWN, (c + 1) * OWN, dtype=np.float32)
        ridx_sh = rows[:, None] - 512.0 * np.arange(JB, dtype=np.float32)[None, :]
        in_maps.append(
            {
                "tok": tokens,
                "qtok": np.ascontiguousarray(tokens[c * OWN : (c + 1) * OWN]),
                "E": E,
                "WqT": WqT,
                "WkT": WkT,
                "WvT": WvT,
                "bq": np.asarray(inputs["bq"], np.float32),
                "bk": np.asarray(inputs["bk"], np.float32),
                "bv": np.asarray(inputs["bv"], np.float32),
                "WpT": np.ascontiguousarray(WpT_pad[:, c * VSH : (c + 1) * VSH]),
                "bp": np.ascontiguousarray(bp_pad[c * VSH : (c + 1) * VSH]),
                "ridx_sh": np.ascontiguousarray(ridx_sh, dtype=np.float32),
            }
        )
    return in_maps


def _run(inputs, trace=False):
    nc = bacc.Bacc(trn_type="TRN2", num_devices=NC, debug=False)
    build(nc)
    nc.compile()
    in_maps = _prep_inputs(inputs)
    res = run_bass_kernel_spmd(
        nc, in_maps, core_ids=list(range(NC)), trace=trace
    )
    out = np.concatenate(
        [res.results[c]["logits"] for c in range(NC)], axis=1
    )[:, :VOCAB]
    return out, res


def kernel(**inputs) -> np.ndarray:
    out, _ = _run(inputs, trace=False)
    return out


# revision 9
# speedup vs baseline: 1.0272x; 1.0272x over previous
"""Self-contained Trainium2 Bass kernel for nn_CharModel (dense transformer
forward: embed -> single-head causal attention -> vocab projection).

Distribution over 8 NeuronCores:
  - sequence-parallel attention: core c owns tokens [c*512, (c+1)*512)
  - vocab-parallel logits: core c owns padded-vocab columns [c*6400, (c+1)*6400)
  - attention outputs are exchanged with 4 chunked bf16 AllGathers
All matmuls run in bf16 with fp32 PSUM accumulation; softmax stats fp32.
"""
import numpy as np

import concourse.bass as bass
import concourse.mybir as mybir
import concourse.tile as tile
from concourse import bacc
from concourse.bass_utils import run_bass_kernel_spmd
from concourse.masks import make_identity

P = 128
N_TOK = 4096
D = 1024
VOCAB = 50257
NC = 8
VPAD_TOT = 51200  # 50257 padded up to 400*128
VSH = VPAD_TOT // NC  # 6400 per-core vocab shard
OWN = N_TOK // NC  # 512 own tokens
IBLK = OWN // P  # 4 own row-blocks
KT = D // P  # 8 contraction tiles
OT = D // P  # 8 output-feature tiles
CHUNKS = N_TOK // 512  # 8 projection chunks (512 tokens each)
JB = N_TOK // 512  # 8 key strips of 512
JB2 = N_TOK // P  # 32 key tiles of 128
SCALE = 1.0 / 32.0  # 1/sqrt(D)

F32 = mybir.dt.float32
BF16 = mybir.dt.bfloat16
I32 = mybir.dt.int32

# logits v-strips within the 6400-wide shard: 12 x 512 + 1 x 256
VSTRIPS = [(i * 512, 512) for i in range(12)] + [(6144, 256)]


def build(nc: bass.Bass):
    tok = nc.dram_tensor("tok", [N_TOK], I32, kind="ExternalInput")
    qtok = nc.dram_tensor("qtok", [OWN], I32, kind="ExternalInput")
    E = nc.dram_tensor("E", [VOCAB, D], F32, kind="ExternalInput")
    WqT = nc.dram_tensor("WqT", [D, D], F32, kind="ExternalInput")
    WkT = nc.dram_tensor("WkT", [D, D], F32, kind="ExternalInput")
    WvT = nc.dram_tensor("WvT", [D, D], F32, kind="ExternalInput")
    bq = nc.dram_tensor("bq", [D], F32, kind="ExternalInput")
    bk = nc.dram_tensor("bk", [D], F32, kind="ExternalInput")
    bv = nc.dram_tensor("bv", [D], F32, kind="ExternalInput")
    WpT = nc.dram_tensor("WpT", [D, VSH], F32, kind="ExternalInput")
    bp = nc.dram_tensor("bp", [VSH], F32, kind="ExternalInput")
    # ridx_sh[r, jb] = global_row(r) - jb*512, fp32
    ridx_sh = nc.dram_tensor("ridx_sh", [OWN, JB], F32, kind="ExternalInput")
    logits = nc.dram_tensor("logits", [N_TOK, VSH], F32, kind="ExternalOutput")

    with tile.TileContext(nc) as tc:
        with (
            tc.tile_pool(name="const", bufs=1) as const,
            tc.tile_pool(name="dram", bufs=1, space="DRAM") as dram,
        ):
            ident = const.tile([P, P], BF16)
            make_identity(nc, ident[:])

            bv_bc = const.tile([P, D], F32)
            nc.sync.dma_start(bv_bc[:], bv.ap()[None, :].to_broadcast([P, D]))

            bq_t = const.tile([P, OT], F32)
            nc.sync.dma_start(bq_t[:], bq.ap().rearrange("(ot p) -> p ot", p=P))
            bk_t = const.tile([P, OT], F32)
            nc.sync.dma_start(bk_t[:], bk.ap().rearrange("(ot p) -> p ot", p=P))

            rsh = const.tile([P, IBLK, JB], F32)
            nc.sync.dma_start(
                rsh[:], ridx_sh.ap().rearrange("(ib p) jb -> p ib jb", p=P)
            )

            jidx0 = const.tile([P, 512], F32)
            ji = const.tile([P, 512], I32)
            nc.gpsimd.iota(ji[:], pattern=[[1, 512]], base=0, channel_multiplier=0)
            nc.vector.tensor_copy(out=jidx0[:], in_=ji[:])

            tok_sb = const.tile([P, N_TOK // P], I32)
            nc.sync.dma_start(tok_sb[:], tok.ap().rearrange("(g p) -> p g", p=P))
            qtok_sb = const.tile([P, OWN // P], I32)
            nc.sync.dma_start(qtok_sb[:], qtok.ap().rearrange("(g p) -> p g", p=P))

            # DRAM scratch
            Kscr = dram.tile([CHUNKS, P, OT, 512], BF16)
            Vscr = dram.tile([JB2, P, D], BF16)
            oTb = [dram.tile([P, KT, P], BF16, name=f"oTb{q}") for q in range(IBLK)]
            gat = [
                dram.tile([NC, P, KT, P], BF16, name=f"gat{q}") for q in range(IBLK)
            ]

            # ---------------- gather + transpose helper ----------------
            def gather_xT(pool, pspool, idx_sb, g0, ngroups, tag):
                """gather token groups [g0, g0+ngroups) -> xT [P, KT, ngroups*P] bf16"""
                xT = pool.tile([P, KT, ngroups * P], BF16, tag=f"xT_{tag}")
                for g in range(ngroups):
                    xg = pool.tile([P, D], F32, tag="xg")
                    nc.gpsimd.indirect_dma_start(
                        out=xg[:],
                        out_offset=None,
                        in_=E.ap(),
                        in_offset=bass.IndirectOffsetOnAxis(
                            ap=idx_sb[:, g0 + g : g0 + g + 1], axis=0
                        ),
                    )
                    xb = pool.tile([P, D], BF16, tag="xb")
                    nc.vector.tensor_copy(out=xb[:], in_=xg[:])
                    for kt in range(KT):
                        pst = pspool.tile([P, P], BF16, tag="ptr")
                        nc.tensor.transpose(
                            pst[:], xb[:, kt * P : (kt + 1) * P], ident[:]
                        )
                        nc.vector.tensor_copy(
                            out=xT[:, kt, g * P : (g + 1) * P], in_=pst[:]
                        )
                return xT

            def load_w(pool, dramt, tag):
                wb = pool.tile([P, KT, D], BF16, tag=f"wb_{tag}")
                for half in range(4):
                    wf = pool.tile([P, KT, D // 4], F32, tag="wf")
                    nc.sync.dma_start(
                        wf[:],
                        dramt.ap().rearrange("(kt p) o -> p kt o", p=P)[
                            :, :, half * (D // 4) : (half + 1) * (D // 4)
                        ],
                    )
                    nc.vector.tensor_copy(
                        out=wb[:, :, half * (D // 4) : (half + 1) * (D // 4)],
                        in_=wf[:],
                    )
                return wb

            # ---------------- phase Q: own-token Q projection ----------------
            qT_pool = tc.alloc_tile_pool(name="qT_keep", bufs=1)
            qT = qT_pool.tile([P, OT, OWN], BF16)
            with (
                tc.tile_pool(name="sbq", bufs=2) as sbq,
                tc.tile_pool(name="psq_tr", bufs=2, space="PSUM") as psq_tr,
                tc.tile_pool(name="psq_pp", bufs=4, space="PSUM") as psq_pp,
            ):
                wq_b = load_w(sbq, WqT, "wq")
                xqT = gather_xT(sbq, psq_tr, qtok_sb, 0, OWN // P, "q")
                for ot in range(OT):
                    pp = psq_pp.tile([P, OWN], F32, tag="pp")
                    for kt in range(KT):
                        nc.tensor.matmul(
                            pp[:],
                            lhsT=wq_b[:, kt, ot * P : (ot + 1) * P],
                            rhs=xqT[:, kt, :],
                            start=(kt == 0),
                            stop=(kt == KT - 1),
                        )
                    nc.vector.tensor_scalar(
                        out=qT[:, ot, :],
                        in0=pp[:],
                        scalar1=bq_t[:, ot : ot + 1],
                        scalar2=SCALE,
                        op0=mybir.AluOpType.add,
                        op1=mybir.AluOpType.mult,
                    )

            # ---------------- phase KV: full K/V projections, spill to DRAM ----
            with (
                tc.tile_pool(name="sbkv", bufs=2) as sbkv,
                tc.tile_pool(name="pskv_tr", bufs=2, space="PSUM") as pskv_tr,
                tc.tile_pool(name="pskv_pp", bufs=2, space="PSUM") as pskv_pp,
                tc.tile_pool(name="pskv_pv", bufs=2, space="PSUM") as pskv_pv,
            ):
                wk_b = load_w(sbkv, WkT, "wk")
                wv_b = load_w(sbkv, WvT, "wv")
                for ch in range(CHUNKS):
                    xT = gather_xT(sbkv, pskv_tr, tok_sb, ch * 4, 4, "kv")
                    # K^T chunk -> Kscr[ch]
                    for ot in range(OT):
                        pk = pskv_pp.tile([P, 512], F32, tag="pp")
                        for kt in range(KT):
                            nc.tensor.matmul(
                                pk[:],
                                lhsT=wk_b[:, kt, ot * P : (ot + 1) * P],
                                rhs=xT[:, kt, :],
                                start=(kt == 0),
                                stop=(kt == KT - 1),
                            )
                        ke = sbkv.tile([P, 512], BF16, tag="ke")
                        nc.vector.tensor_scalar(
                            out=ke[:],
                            in0=pk[:],
                            scalar1=bk_t[:, ot : ot + 1],
                            scalar2=None,
                            op0=mybir.AluOpType.add,
                        )
                        nc.sync.dma_start(Kscr[ch, :, ot, :], ke[:])
                    # V natural chunk -> Vscr[ch*4 + tb]
                    for tb in range(4):
                        pv = pskv_pv.tile([P, D], F32, tag="pv")
                        for kt in range(KT):
                            nc.tensor.matmul(
                                pv[:, 0:512],
                                lhsT=xT[:, kt, tb * P : (tb + 1) * P],
                                rhs=wv_b[:, kt, 0:512],
                                start=(kt == 0),
                                stop=(kt == KT - 1),
                            )
                        for kt in range(KT):
                            nc.tensor.matmul(
                                pv[:, 512:1024],
                                lhsT=xT[:, kt, tb * P : (tb + 1) * P],
                                rhs=wv_b[:, kt, 512:1024],
                                start=(kt == 0),
                                stop=(kt == KT - 1),
                            )
                        ve = sbkv.tile([P, D], BF16, tag="ve")
                        nc.vector.tensor_copy(out=ve[:], in_=pv[:])
                        nc.sync.dma_start(Vscr[ch * 4 + tb, :, :], ve[:])

            # ---------------- phase attention (own rows) ----------------------
            # WpT load/cast pieces are interleaved between attention steps so
            # the Sync/Vector streams never block on a monolithic 25MB load.
            wp_pool = tc.alloc_tile_pool(name="wp_keep", bufs=1)
            wp_b = wp_pool.tile([P, KT, VSH], BF16)
            wp_pieces = [(kt, h) for kt in range(KT) for h in range(8)]
            WPW = VSH // 8  # 800-wide load/cast pieces

            with (
                tc.tile_pool(name="sbat", bufs=2) as sbat,
                tc.tile_pool(name="sbkj", bufs=3) as sbkj,
                tc.tile_pool(name="psat_sc", bufs=3, space="PSUM") as ps_sc,
                tc.tile_pool(name="psat_av", bufs=1, space="PSUM") as ps_av,
                tc.tile_pool(name="psat_tr", bufs=2, space="PSUM") as ps_tr,
            ):

                def load_wp_piece(i):
                    if i >= len(wp_pieces):
                        return
                    kt, half = wp_pieces[i]
                    v0 = half * WPW
                    v1 = (half + 1) * WPW
                    wpf = sbat.tile([P, WPW], F32, tag="wpf")
                    nc.sync.dma_start(
                        wpf[:],
                        WpT.ap().rearrange("(kt p) v -> p kt v", p=P)[:, kt, v0:v1],
                    )
                    nc.vector.tensor_copy(out=wp_b[:, kt, v0:v1], in_=wpf[:])

                wp_i = 0
                for ib in range(IBLK):
                    a_row = sbat.tile([P, N_TOK], BF16, tag="a_row")
                    for jb in range(JB):
                        load_wp_piece(wp_i)
                        load_wp_piece(wp_i + 1)
                        wp_i += 2
                        kjb = sbkj.tile([P, OT, 512], BF16, tag="kjb")
                        nc.sync.dma_start(kjb[:], Kscr[jb, :, :, :])
                        ps = ps_sc.tile([P, 512], F32, tag="sc")
                        for ot in range(OT):
                            nc.tensor.matmul(
                                ps[:],
                                lhsT=qT[:, ot, ib * P : (ib + 1) * P],
                                rhs=kjb[:, ot, :],
                                start=(ot == 0),
                                stop=(ot == OT - 1),
                            )
                        astr = a_row[:, jb * 512 : (jb + 1) * 512]
                        nc.scalar.activation(
                            astr, ps[:], mybir.ActivationFunctionType.Exp
                        )
                        # multiply by causal mask: (jidx0 <= ridx - jb*512) * exp
                        nc.vector.scalar_tensor_tensor(
                            out=astr,
                            in0=jidx0[:],
                            scalar=rsh[:, ib, jb : jb + 1],
                            in1=astr,
                            op0=mybir.AluOpType.is_le,
                            op1=mybir.AluOpType.mult,
                        )
                    dsum = sbat.tile([P, 1], F32, tag="dsum")
                    nc.vector.tensor_reduce(
                        out=dsum[:],
                        in_=a_row[:],
                        axis=mybir.AxisListType.X,
                        op=mybir.AluOpType.add,
                    )
                    rden = sbat.tile([P, 1], F32, tag="rden")
                    nc.vector.reciprocal(rden[:], dsum[:])

                    pav = ps_av.tile([P, D], F32, tag="av")
                    for j2 in range(JB2):
                        pat = ps_tr.tile([P, P], BF16, tag="tr")
                        nc.tensor.transpose(
                            pat[:], a_row[:, j2 * P : (j2 + 1) * P], ident[:]
                        )
                        at = sbat.tile([P, P], BF16, tag="at")
                        nc.vector.tensor_copy(out=at[:], in_=pat[:])
                        vj = sbat.tile([P, D], BF16, tag="vj")
                        nc.sync.dma_start(vj[:], Vscr[j2, :, :])
                        nc.tensor.matmul(
                            pav[:, 0:512],
                            lhsT=at[:],
                            rhs=vj[:, 0:512],
                            start=(j2 == 0),
                            stop=(j2 == JB2 - 1),
                        )
                        nc.tensor.matmul(
                            pav[:, 512:1024],
                            lhsT=at[:],
                            rhs=vj[:, 512:1024],
                            start=(j2 == 0),
                            stop=(j2 == JB2 - 1),
                        )
                    o_bf = sbat.tile([P, D], BF16, tag="o_bf")
                    nc.vector.scalar_tensor_tensor(
                        out=o_bf[:],
                        in0=pav[:],
                        scalar=rden[:, :1],
                        in1=bv_bc[:],
                        op0=mybir.AluOpType.mult,
                        op1=mybir.AluOpType.add,
                    )
                    oT = sbat.tile([P, KT, P], BF16, tag="oT")
                    for kt in range(KT):
                        pot = ps_tr.tile([P, P], BF16, tag="tr")
                        nc.tensor.transpose(
                            pot[:], o_bf[:, kt * P : (kt + 1) * P], ident[:]
                        )
                        nc.vector.tensor_copy(out=oT[:, kt, :], in_=pot[:])
                    nc.sync.dma_start(oTb[ib][:], oT[:])
                    nc.gpsimd.collective_compute(
                        "AllGather",
                        mybir.AluOpType.bypass,
                        replica_groups=[list(range(NC))],
                        ins=[oTb[ib].opt()],
                        outs=[gat[ib].opt()],
                    )

            # ---------------- phase logits ------------------------------------
            with (
                tc.tile_pool(name="sblg", bufs=3) as sblg,
                tc.tile_pool(name="sbbp", bufs=1) as sbbp,
                tc.tile_pool(name="pslg", bufs=6, space="PSUM") as pslg,
            ):
                bp_bc = sbbp.tile([P, VSH], F32)
                nc.sync.dma_start(bp_bc[:], bp.ap()[None, :].to_broadcast([P, VSH]))
                for q in range(IBLK):
                    for c in range(NC):
                        ibg = c * IBLK + q  # global row-block
                        lt = sblg.tile([P, KT, P], BF16, tag="lt")
                        nc.sync.dma_start(lt[:], gat[q][c, :, :, :])
                        for v0, vw in VSTRIPS:
                            pl = pslg.tile([P, 512], F32, tag="lg")
                            for kt in range(KT):
                                nc.tensor.matmul(
                                    pl[:, :vw],
                                    lhsT=lt[:, kt, :],
                                    rhs=wp_b[:, kt, v0 : v0 + vw],
                                    start=(kt == 0),
                                    stop=(kt == KT - 1),
                                )
                            lo = sblg.tile([P, 512], F32, tag="lo")
                            nc.vector.tensor_tensor(
                                out=lo[:, :vw],
                                in0=pl[:, :vw],
                                in1=bp_bc[:, v0 : v0 + vw],
                                op=mybir.AluOpType.add,
                            )
                            nc.sync.dma_start(
                                logits.ap()[
                                    ibg * P : (ibg + 1) * P, v0 : v0 + vw
                                ],
                                lo[:, :vw],
                            )
            wp_pool.release()
            qT_pool.release()
    return nc


def _prep_inputs(inputs):
    """Host-side shard prep: slicing, transposes, padding only."""
    tokens = np.ascontiguousarray(np.asarray(inputs["tokens"]).astype(np.int32))
    E = np.asarray(inputs["E"], dtype=np.float32)
    WqT = np.ascontiguousarray(np.asarray(inputs["Wq"], np.float32).T)
    WkT = np.ascontiguousarray(np.asarray(inputs["Wk"], np.float32).T)
    WvT = np.ascontiguousarray(np.asarray(inputs["Wv"], np.float32).T)
    Wp = np.asarray(inputs["Wp"], np.float32)
    WpT_pad = np.zeros((D, VPAD_TOT), np.float32)
    WpT_pad[:, :VOCAB] = Wp.T
    bp_pad = np.zeros((VPAD_TOT,), np.float32)
    bp_pad[:VOCAB] = np.asarray(inputs["bp"], np.float32)

    in_maps = []
    for c in range(NC):
        rows = np.arange(c * OWN, (c + 1) * OWN, dtype=np.float32)
        ridx_sh = rows[:, None] - 512.0 * np.arange(JB, dtype=np.float32)[None, :]
        in_maps.append(
            {
                "tok": tokens,
                "qtok": np.ascontiguousarray(tokens[c * OWN : (c + 1) * OWN]),
                "E": E,
                "WqT": WqT,
                "WkT": WkT,
                "WvT": WvT,
                "bq": np.asarray(inputs["bq"], np.float32),
                "bk": np.asarray(inputs["bk"], np.float32),
                "bv": np.asarray(inputs["bv"], np.float32),
                "WpT": np.ascontiguousarray(WpT_pad[:, c * VSH : (c + 1) * VSH]),
                "bp": np.ascontiguousarray(bp_pad[c * VSH : (c + 1) * VSH]),
                "ridx_sh": np.ascontiguousarray(ridx_sh, dtype=np.float32),
            }
        )
    return in_maps


def _run(inputs, trace=False):
    nc = bacc.Bacc(trn_type="TRN2", num_devices=NC, debug=False)
    build(nc)
    nc.compile()
    in_maps = _prep_inputs(inputs)
    res = run_bass_kernel_spmd(
        nc, in_maps, core_ids=list(range(NC)), trace=trace
    )
    out = np.concatenate(
        [res.results[c]["logits"] for c in range(NC)], axis=1
    )[:, :VOCAB]
    return out, res


def kernel(**inputs) -> np.ndarray:
    out, _ = _run(inputs, trace=False)
    return out
